# revision 64
# baseline (speedup 1.0000x reference)
"""Trainium2 Bass kernel for nn_Attention_927712936452.

Two-branch attention (self branch over x, cross branch of y-queries over
concat(x,y) keys/values), QKV + output projection, H=12 heads of 64.

Distribution: pure data-parallel over batch B=8 across the 8 NeuronCores
(one batch element per core, weights replicated). No collectives.

Per-core layout strategy (all matmul operands bf16, fp32 PSUM accumulate):
  - host supplies z^T [C, S] (c on partitions) so QKV needs no device
    transpose
  - stage 1 produces q^T/k^T [C, S] (head-pair per 128-row tile) and
    v-hat [S, H*65] where every head gets a 65-wide [v_h | 1] block
  - scores are computed in [k, q] layout per (head-pair, kt-pair): the two
    heads go to disjoint 64-row PE groups (tile_position) and different
    PSUM banks so the pair can issue adjacently; one ScalarE exp with
    fused 1/8 scale covers a whole [128, 1024] two-kt two-head PSUM tile
  - AV is *query-stationary*: lhsT = a [128-key, 128-query] chunk of the
    exp'd probabilities, moving operand = the head's [v_h | 1] block
    (N=65).  The output lands naturally as [query, head-dim] and the
    ones column puts the softmax denominator in the accumulator at
    column 64 *per query partition*, so normalization is a plain
    per-partition tensor_scalar multiply - no cross-partition broadcast.
  - normalized per-qchunk [128, C] activations are PE-transposed (6
    identity matmuls) into [c, q] tiles that feed the output projection
    as lhsT
  - emission: cross branch hp-outer (stage-1 K/Q chains of hp+1 spread as
    filler; V chains just-in-time inside hp0), self branch qb-outer with
    the deferred transpose+projection blocks dripped in ~0.4us pieces
    through a global queue, so the PE fills the ACT-bound stretches and
    ScalarE (the exp engine, ~300us of work) never starves.  Each
    (hp, qb) block's last AV pair + normalization are carried into the
    next block's emission so an exp-blocked AV never sits ahead of the
    next score matmuls in the in-order PE stream.

  Two scheduling-correctness rules learned the hard way:
  - emission IS dependency order: a consumer emitted before its producer
    gets no dependency edge (hp0's own K chains must be emitted inside
    pass 1, before the score matmuls that read them)
  - a start=True matmul pending-zeroes its whole 2KB PSUM bank, so the
    four AV accumulation slices sharing a bank are initialized by one
    always-ready K=1 zero-matmul and accumulate with start=False.
"""

import numpy as np

try:
    import concourse.bass as bass  # noqa: F401
except ImportError:
    import sys

    sys.path.insert(0, "/opt/trn_rl_repo")

import ml_dtypes
from contextlib import ExitStack

import concourse.bass as bass
import concourse.tile as tile
from concourse import bacc, bass_utils, mybir

BF = mybir.dt.bfloat16
F32 = mybir.dt.float32
EXP = mybir.ActivationFunctionType.Exp

# Full-size problem constants
B = 8
N_FULL = 1024  # x sequence length (self branch queries/keys)
L_FULL = 1024  # y sequence length (cross branch queries)
C_FULL = 768
H_FULL = 12
DH = 64


def build_nc(C=C_FULL, N=N_FULL, L=L_FULL, qw=256, ablate=(), small_out=False, taps=False):
    """Build the per-core Bass graph.

    C: model dim (multiple of 128, heads = C//64, head pairs = C//128)
    N: x length, L: y length (each a multiple of qw; qw multiple of 128)
    """
    S = N + L
    CT = C // 128  # head-pair tiles / c-tiles
    NKT = S // 128  # k tiles over full sequence
    NKT_SELF = N // 128  # k tiles for self branch
    CH = C // 2  # proj N-split (PSUM bank limit: <=512 fp32)
    QC = qw // 128  # 128-query chunks per q block
    assert CH <= 512 and qw % 128 == 0 and N % qw == 0 and L % qw == 0
    assert NKT % 2 == 0 and NKT_SELF % 2 == 0
    scale = DH ** -0.5

    nc = bacc.Bacc("TRN2", target_bir_lowering=False, debug=False)
    zt_d = nc.dram_tensor("z_t", [C, S], BF, kind="ExternalInput")
    wq_d = nc.dram_tensor("qkv_wt", [C, 3 * C], BF, kind="ExternalInput")
    pw_d = nc.dram_tensor("proj_wt", [C, C], BF, kind="ExternalInput")
    pb_d = nc.dram_tensor("proj_b", [1, C], F32, kind="ExternalInput")
    id_d = nc.dram_tensor("ident", [128, 128], BF, kind="ExternalInput")
    on = 128 if small_out else N
    ol = 128 if small_out else L
    xo_d = nc.dram_tensor("x_out", [on, C], BF, kind="ExternalOutput")
    yo_d = nc.dram_tensor("y_out", [ol, C], BF, kind="ExternalOutput")
    if taps:
        dbg_p2c = nc.dram_tensor("dbg_p2c", [128, 4 * qw], F32, kind="ExternalOutput")
        dbg_accc = nc.dram_tensor("dbg_accc", [128, (qw // 128) * 130], F32, kind="ExternalOutput")
        dbg_v8 = nc.dram_tensor("dbg_v8", [128, (C // 64) * 65], F32, kind="ExternalOutput")
        dbg_att0 = nc.dram_tensor("dbg_att0", [128, 128], F32, kind="ExternalOutput")
        dbg_rcp = nc.dram_tensor("dbg_rcp", [128, 8], F32, kind="ExternalOutput")
        dbg_ktt = nc.dram_tensor("dbg_ktt", [128, N + L], F32, kind="ExternalOutput")
        dbg_qtt = nc.dram_tensor("dbg_qtt", [128, N + L], F32, kind="ExternalOutput")
        dbg_v = nc.dram_tensor("dbg_v", [128, (C // 64) * 65], F32, kind="ExternalOutput")
        dbg_p2 = nc.dram_tensor("dbg_p2", [128, 4 * qw], F32, kind="ExternalOutput")
        dbg_acc = nc.dram_tensor("dbg_acc", [128, (qw // 128) * 130], F32, kind="ExternalOutput")
        dbg_attn = nc.dram_tensor("dbg_attn", [128, C], F32, kind="ExternalOutput")

    with tile.TileContext(nc) as tc, ExitStack() as ctx:
        zt_p = ctx.enter_context(tc.tile_pool(name="zt", bufs=1))
        wq_p = ctx.enter_context(tc.tile_pool(name="wq", bufs=1))
        qt_p = ctx.enter_context(tc.tile_pool(name="qt", bufs=CT))
        kt_p = ctx.enter_context(tc.tile_pool(name="kt", bufs=CT))
        v_p = ctx.enter_context(tc.tile_pool(name="v", bufs=NKT))
        pw_p = ctx.enter_context(tc.tile_pool(name="pw", bufs=1))
        misc_p = ctx.enter_context(tc.tile_pool(name="misc", bufs=1))
        p2_p = ctx.enter_context(tc.tile_pool(name="p2", bufs=12))
        attn_p = ctx.enter_context(tc.tile_pool(name="attn", bufs=12))
        tsb_p = ctx.enter_context(tc.tile_pool(name="tsb", bufs=2))
        rcp_p = ctx.enter_context(tc.tile_pool(name="rcp", bufs=4))
        out_p = ctx.enter_context(tc.tile_pool(name="osb", bufs=3))
        spsum = ctx.enter_context(tc.tile_pool(name="spsum", bufs=2, space="PSUM"))
        accp = ctx.enter_context(tc.tile_pool(name="accp", bufs=2, space="PSUM"))
        chp = ctx.enter_context(tc.tile_pool(name="chp", bufs=2, space="PSUM"))

        # ---- input loads ----
        # All operand tensors live as single wide SBUF tiles (c-tiles side
        # by side on the free axis) so the whole input set needs only ~9
        # large DMAs on one queue, ordered so the first-exp critical path
        # (chain K0: zt cols 0:512 + wq cols C:C+128; chain Q2: zt cols
        # 1024:1536 + wq cols 0:128) lands first.  The ACT queue stays
        # clean for the exp stream.
        zt_t = zt_p.tile([128, CT * S], BF, tag="zt", name="zt")
        wq_t = wq_p.tile([128, CT * 3 * C], BF, tag="wq", name="wq")
        pw_t = pw_p.tile([128, CT * C], BF, tag="pw", name="pw")
        zt = [zt_t[:, c * S : (c + 1) * S] for c in range(CT)]
        wq = [wq_t[:, c * 3 * C : (c + 1) * 3 * C] for c in range(CT)]
        pw = [pw_t[:, c * C : (c + 1) * C] for c in range(CT)]

        def dma2(queue, dst, src_ap, c, period, nblk, off, w):
            # column chunks [off:off+w] of the first nblk period-wide blocks
            d = dst.rearrange("p (k x) -> p k x", x=period)[:, 0:nblk, off : off + w]
            s = src_ap[c * 128 : (c + 1) * 128, :].rearrange(
                "p (k x) -> p k x", x=period
            )[:, 0:nblk, off : off + w]
            queue.dma_start(d, s)

        if S != 2048:
            # debug/small sizes: plain per-tile loads
            for c in range(CT):
                nc.sync.dma_start(zt[c], zt_d.ap()[c * 128 : (c + 1) * 128, :])
                nc.sync.dma_start(wq[c], wq_d.ap()[c * 128 : (c + 1) * 128, :])
            nc.sync.dma_start(
                pw_t[:].rearrange("p (b s) -> p b s", b=CT),
                pw_d.ap().rearrange("(b p) s -> p b s", p=128),
            )
            bias = misc_p.tile([128, C], F32, tag="bias")
            nc.sync.dma_start(bias[:], pb_d.ap().to_broadcast((128, C)))
            ident = misc_p.tile([128, 128], BF, tag="ident")
            nc.sync.dma_start(ident[:], id_d.ap())
            skip_main_loads = True
        else:
            skip_main_loads = False
        # 1: zt cols 0:512 + 1024:1536 (self keys kt0-3, cross qb0/1
        # queries) on SP; hp0's Q (cols 0:128) and K (cols C:C+128) weight
        # columns on ACT (done issuing before the first exp arrives there)
        for c in range(CT if not skip_main_loads else 0):
            dma2(nc.sync if c % 2 == 0 else nc.gpsimd, zt[c], zt_d.ap(), c, 1024, 2, 0, 512)
        for c in range(CT if not skip_main_loads else 0):
            dma2(nc.scalar, wq[c], wq_d.ap(), c, C, 2, 0, 128)
        # 2: V weight block + rest of zt
        for c in range(CT if not skip_main_loads else 0):
            nc.gpsimd.dma_start(
                wq[c][:, 2 * C : 3 * C],
                wq_d.ap()[c * 128 : (c + 1) * 128, 2 * C : 3 * C],
            )
        for c in range(CT if not skip_main_loads else 0):
            dma2(nc.sync, zt[c], zt_d.ap(), c, 1024, 2, 512, 512)
        # 3: remaining Q/K weight columns (hp1+ chains, filler work) and
        # head-out-only operands
        for c in range(CT if not skip_main_loads else 0):
            dma2(nc.gpsimd, wq[c], wq_d.ap(), c, C, 2, 128, C - 128)
        if not skip_main_loads:
            nc.sync.dma_start(
                pw_t[:].rearrange("p (b s) -> p b s", b=CT),
                pw_d.ap().rearrange("(b p) s -> p b s", p=128),
            )
            bias = misc_p.tile([128, C], F32, tag="bias")
            nc.sync.dma_start(bias[:], pb_d.ap().to_broadcast((128, C)))
            ident = misc_p.tile([128, 128], BF, tag="ident")
            nc.sync.dma_start(ident[:], id_d.ap())
        # constant-zero operands for the PE-side acc-clearing matmul
        zl = misc_p.tile([1, 128], BF, tag="zl")
        nc.vector.memset(zl[:], 0.0)
        zr = misc_p.tile([1, 2 * QC * 65], BF, tag="zr")
        nc.vector.memset(zr[:], 0.0)
        # warm the PE during the input-DMA wait: ~4us of dummy matmuls
        # ramp the p-state (model) / HAM clock gate (hardware) so the
        # first real chains run at full rate.  Input-independent: only
        # needs the zero tiles above.
        warm = chp.tile([128, 512], F32, tag="chain", name="pewarm")
        for _ in range(16):
            nc.tensor.matmul(
                warm[:, 0 : 2 * QC * 65], zl[:], zr[:],
                start=True, stop=True, skip_group_check=True,
            )

        # ---- stage 1: V = z @ Wv ----
        # v-hat layout: every head gets a 65-wide block [v_h | 1]; the ones
        # column makes the query-stationary AV matmul (N=65) deposit the
        # softmax denominator in accumulator column 64.
        H = C // DH
        HH = H // 2  # heads per CH half
        v_sb = [v_p.tile([128, H * 65], BF, tag="v", name=f"v{i}") for i in range(NKT)]
        v_emitted = set()

        def emit_v(st):
            if st in v_emitted:
                return
            v_emitted.add(st)
            vh3 = v_sb[st][:].rearrange("p (h e) -> p h e", e=65)
            nc.vector.memset(vh3[:, :, 64:65], 1.0)
            for vn in range(2):
                ps = chp.tile([128, 512], F32, tag="chain", name=f"vps{st}_{vn}")
                for c in range(CT):
                    nc.tensor.matmul(
                        ps[:, 0:CH],
                        zt[c][:, st * 128 : (st + 1) * 128],
                        wq[c][:, 2 * C + vn * CH : 2 * C + (vn + 1) * CH],
                        start=(c == 0),
                        stop=(c == CT - 1),
                    )
                nc.vector.tensor_copy(
                    vh3[:, vn * HH : (vn + 1) * HH, 0:64],
                    ps[:, 0:CH].rearrange("p (h e) -> p h e", e=64),
                )

        # ---- stage 1: K^T, Q^T  ([d, s] layout, head pair per tile) ----
        qtt = [qt_p.tile([128, S], BF, tag="qt", name=f"qtt{i}") for i in range(CT)]
        ktt = [kt_p.tile([128, S], BF, tag="kt", name=f"ktt{i}") for i in range(CT)]

        def emit_kq_chain(t, n, dst, dbase):
            ps = chp.tile([128, 512], F32, tag="chain", name=f"kq{t}_{n}_{dbase}")
            for c in range(CT):
                nc.tensor.matmul(
                    ps[:],
                    wq[c][:, dbase + t * 128 : dbase + (t + 1) * 128],
                    zt[c][:, n * 512 : (n + 1) * 512],
                    start=(c == 0),
                    stop=(c == CT - 1),
                )
            nc.vector.tensor_copy(dst[t][:, n * 512 : (n + 1) * 512], ps[:])

        # chain order: K chunks early (kt pairs consume them in order),
        # cross-query chunks before self-query chunks (cross runs first)
        KQ_ORDER = [
            (ktt, C, 0),
            (qtt, 0, 2),
            (ktt, C, 1),
            (ktt, C, 2),
            (ktt, C, 3),
            (qtt, 0, 3),
            (qtt, 0, 0),
            (qtt, 0, 1),
        ]

        def kq_chain_thunks(t):
            return [
                (lambda t=t, n=n, dst=dst, dbase=dbase: emit_kq_chain(t, n, dst, dbase))
                for dst, dbase, n in KQ_ORDER
            ]

        # ---- attention: scores + exp + query-stationary AV ----
        drip = []  # fine-grained deferred work (head-out pieces)

        def emit_attn_hp(branch, hp, qb, att_n, filler=(), carry=None, pre_fill=()):
            """Scores+exp+AV for one (branch, head-pair, q-block).

            att_n: list of QC per-qchunk [128, C] bf16 tiles this hp fills
            at columns [hp*128, hp*128+128).
            """
            filler = list(filler)
            nkt = NKT_SELF if branch == 0 else NKT
            npair = nkt // 2
            qbase = 0 if branch == 0 else N
            qoff = qbase + qb * qw
            # acc layout: [qchunk][head] 65-wide blocks.  PSUM accumulation
            # groups share this bank, and a start=True matmul marks the
            # whole 2KB zero-region pending-zero (wiping sibling groups'
            # first contribution) - so instead zero the tile explicitly and
            # accumulate with start=False throughout.
            acc = accp.tile(
                [128, QC * 2 * 65], F32, tag="acc", name=f"acc{branch}{hp}{qb}"
            )
            # Zero acc with a K=1 PE matmul (start=True, full-region
            # write): PE executes its stream in order and this matmul is
            # always ready, so no AV can race ahead of it - unlike a
            # cross-engine DVE memset.  A start=True on an AV matmul
            # instead would pending-zero the whole shared bank and wipe
            # sibling slices' first contributions.
            nc.tensor.matmul(acc[:], zl[:], zr[:], start=True, stop=True,
                             skip_group_check=True)
            # pass 1: all scores + exp for this (hp, qb) — these feed the
            # ACT bottleneck and must outrank every other PE instruction
            p2s = []
            pre_fill = list(pre_fill)
            for i in range(npair):
                # chains consumed by THIS pass-1 (e.g. hp0's own K chunks)
                # must be emitted before the score matmuls that read them:
                # a reader emitted before its writer gets no dependency.
                if pre_fill and i >= 2 and i % 2 == 0:
                    pre_fill.pop(0)()
                s2 = spsum.tile(
                    [128, 4 * qw], F32, tag="s2", name=f"s2_{branch}{hp}{qb}{i}"
                )
                # col layout [ktA_hA | ktB_hA | ktA_hB | ktB_hB]: the
                # (hA, hB) tile-position pair of one kt targets different
                # PSUM banks and can run concurrently in the PE array
                for a in range(2):
                    kt = 2 * i + a
                    for m in range(2):
                        nc.tensor.matmul(
                            s2[:, (2 * m + a) * qw : (2 * m + a + 1) * qw],
                            ktt[hp][m * 64 : (m + 1) * 64, kt * 128 : (kt + 1) * 128],
                            qtt[hp][m * 64 : (m + 1) * 64, qoff : qoff + qw],
                            start=True,
                            stop=True,
                            tile_position=(m * 64, 0),
                        )
                p2 = p2_p.tile([128, 4 * qw], BF, tag="p2", name=f"p2_{branch}{hp}{qb}{i}")
                nc.scalar.activation(p2[:], s2[:], EXP, scale=scale)
                p2s.append(p2)
                if taps and branch == 0 and hp == 0 and qb == 0 and i == 0:
                    nc.gpsimd.dma_start(dbg_p2.ap(), p2[:])
                if taps and branch == 1 and hp == 0 and qb == 0 and i == 4:
                    nc.gpsimd.dma_start(dbg_p2c.ap(), p2[:])
                if carry is not None and i == min(2, npair - 1):
                    carry()
                    carry = None
                elif drip:
                    drip.pop(0)()
            def emit_av(i):
                p2 = p2s[i]
                for a in range(2):
                    kt = 2 * i + a
                    emit_v(kt)
                    first = kt == 0
                    last = kt == nkt - 1
                    for j in range(QC):
                        for m in range(2):
                            # a start=True matmul pending-zeroes the WHOLE
                            # 2KB psum bank (all four slices), and PE
                            # executes matmuls pc-monotone - so only the
                            # block's very first AV clears; every other
                            # slice's first write consumes its pending-zero
                            # bytes (overwrite-init), later kts accumulate.
                            nc.tensor.matmul(
                                acc[:, j * 130 + m * 65 : j * 130 + m * 65 + 65],
                                p2[:, (2 * m + a) * qw + j * 128 : (2 * m + a) * qw + (j + 1) * 128],
                                v_sb[kt][:, (2 * hp + m) * 65 : (2 * hp + m + 1) * 65],
                                start=False,
                                stop=False,
                                skip_group_check=True,
                            )

            # pass 2: V tiles just-in-time, then the query-stationary AVs.
            # The last pair + normalization are returned as a tail thunk the
            # caller emits inside the NEXT block, so an exp-blocked AV never
            # sits (in-order) ahead of the next block's score matmuls.
            for i in range(npair - 1):
                # filler BEFORE the AV group: the AVs may park on their exp
                # semaphore (4-deep wait window), so give the engine issued
                # work to chew on during the wait
                if filler:
                    filler.pop(0)()
                elif drip:
                    drip.pop(0)()
                emit_av(i)

            def tail():
                emit_av(npair - 1)
                if taps and branch == 0 and hp == 0 and qb == 0:
                    tmp = out_p.tile([128, 2 * C], BF, tag="osb", name="dbgacc")
                    tmpf = tmp[:, 0 : 2 * QC * 130].bitcast(F32)
                    nc.vector.tensor_copy(tmpf[:, 0 : QC * 130], acc[:])
                    nc.sync.dma_start(dbg_acc.ap(), tmpf[:, 0 : QC * 130])
                if taps and branch == 1 and hp == 0 and qb == 0:
                    tmpc = out_p.tile([128, 2 * C], BF, tag="osb", name="dbgaccc")
                    tmpcf = tmpc[:, 0 : 2 * QC * 130].bitcast(F32)
                    nc.vector.tensor_copy(tmpcf[:, 0 : QC * 130], acc[:])
                    nc.sync.dma_start(dbg_accc.ap(), tmpcf[:, 0 : QC * 130])
                    nc.gpsimd.dma_start(dbg_v8.ap(), v_sb[8][:])
                for th in filler:
                    th()
                # normalize: per-partition softmax divide, no broadcast
                acc3 = acc[:].rearrange("p (g e) -> p g e", e=65)
                rcp = rcp_p.tile(
                    [128, QC * 2], F32, tag="rcp", name=f"rcp{branch}{hp}{qb}"
                )
                nc.vector.reciprocal(rcp[:], acc3[:, :, 64:65])
                if taps and branch == 0 and hp == 0 and qb == 0:
                    nc.sync.dma_start(dbg_rcp.ap()[:, 0 : QC * 2], rcp[:])
                if "nodiv" in ablate:
                    for j in range(QC):
                        nc.vector.tensor_copy(
                            att_n[j][:, hp * 128 : hp * 128 + 128],
                            acc3[:, 2 * j : 2 * j + 2, 0:64],
                        )
                    return
                for j in range(QC):
                    for m in range(2):
                        nc.vector.tensor_scalar_mul(
                            att_n[j][:, (2 * hp + m) * 64 : (2 * hp + m + 1) * 64],
                            acc[:, j * 130 + m * 65 : j * 130 + m * 65 + 64],
                            rcp[:, 2 * j + m : 2 * j + m + 1],
                        )
                if taps and branch == 0 and hp == 0 and qb == 0:
                    nc.gpsimd.dma_start(
                        dbg_att0.ap(), att_n[0][:, 0:128]
                    )

            return tail

        # ---- transpose + projection + store for one q block ----
        # split into ~0.7us thunks so head-out work can drip into pass-2
        # filler slots without delaying the next block's score matmuls
        def headout_thunks(branch, qb, att_n):
            out_d = xo_d if branch == 0 else yo_d
            thunks = []
            for j in range(QC):
                state = {}

                def t_transp(j=j, state=state):
                    tsb = tsb_p.tile(
                        [128, C], BF, tag="tsb", name=f"tsb{branch}{qb}{j}"
                    )
                    ps = chp.tile(
                        [128, 1024], BF, tag="chain", name=f"tp{branch}{qb}{j}"
                    )
                    for k in range(CT):
                        nc.tensor.transpose(
                            ps[:, k * 128 : (k + 1) * 128],
                            att_n[j][:, k * 128 : (k + 1) * 128],
                            ident[:],
                        )
                    nc.vector.tensor_copy(tsb[:], ps[:, 0 : CT * 128])
                    state["tsb"] = tsb
                    state["osb"] = out_p.tile(
                        [128, C], BF, tag="osb", name=f"osb{branch}{qb}{j}"
                    )

                def t_proj(half, lo, hi, j=j, state=state):
                    # piece [lo:hi) of the 6-matmul accumulation chain
                    tsb, osb = state["tsb"], state["osb"]
                    if lo == 0:
                        state[half] = chp.tile(
                            [128, 512], F32, tag="chain",
                            name=f"pp{branch}{qb}{j}{half}",
                        )
                    pp = state[half]
                    for ct in range(lo, hi):
                        nc.tensor.matmul(
                            pp[:, 0:CH],
                            tsb[:, ct * 128 : (ct + 1) * 128],
                            pw[ct][:, half * CH : (half + 1) * CH],
                            start=(ct == 0),
                            stop=(ct == CT - 1),
                        )
                    if hi < CT:
                        return
                    nc.vector.tensor_add(
                        osb[:, half * CH : (half + 1) * CH],
                        pp[:, 0:CH],
                        bias[:, half * CH : (half + 1) * CH],
                    )
                    if half == 1:
                        row0 = qb * qw + j * 128
                        if small_out:
                            if row0 == 0:
                                nc.sync.dma_start(out_d.ap()[0:128, :], osb[:])
                        else:
                            nc.sync.dma_start(
                                out_d.ap()[row0 : row0 + 128, :], osb[:]
                            )

                thunks += [
                    t_transp,
                    lambda t=t_proj: t(0, 0, 3),
                    lambda t=t_proj: t(0, 3, CT),
                    lambda t=t_proj: t(1, 0, 3),
                    lambda t=t_proj: t(1, 3, CT),
                ]
            return thunks

        def emit_headout(branch, qb, att_n):
            for th in headout_thunks(branch, qb, att_n):
                th()

        def alloc_attn(tagix):
            return [
                attn_p.tile([128, C], BF, tag="attn", name=f"attn{tagix}_{j}")
                for j in range(QC)
            ]

        # ---- cross branch: hp-outer so stage-1 K/Q chains stream as filler
        nq_cross = L // qw
        att_cross = [alloc_attn(f"c{q}") for q in range(nq_cross)]
        carry = None
        for hp in range(CT):
            if hp == 0:
                # prologue: K0 and Q2 chains interleaved c-by-c (each MM
                # unblocks on its own zt chunk DMA), rest of hp0's chains
                # as filler inside qb0/1
                ps_k = chp.tile([128, 512], F32, tag="chain", name="kq_pro_k")
                ps_q = chp.tile([128, 512], F32, tag="chain", name="kq_pro_q")
                for c in range(CT):
                    for ps, dbase, n in ((ps_k, C, 0), (ps_q, 0, 2)):
                        nc.tensor.matmul(
                            ps[:],
                            wq[c][:, dbase : dbase + 128],
                            zt[c][:, n * 512 : (n + 1) * 512],
                            start=(c == 0),
                            stop=(c == CT - 1),
                        )
                nc.vector.tensor_copy(ktt[0][:, 0:512], ps_k[:])
                nc.vector.tensor_copy(qtt[0][:, 1024:1536], ps_q[:])
                rest0 = kq_chain_thunks(0)[2:]
            if hp + 1 < CT:
                thunks = kq_chain_thunks(hp + 1)
                if hp + 1 == CT - 1:
                    # hp5's self-branch Q chains (consumed only deep in the
                    # self phase) become hp5's own boundary filler
                    reserve = thunks[6:]
                    thunks = thunks[:6]
            else:
                thunks = reserve
            if hp == CT - 1:
                att_self_early = [alloc_attn("s0"), alloc_attn("s1")]
            for qb in range(nq_cross):
                pre = ()
                if hp == 0:
                    if qb == 0:
                        pre = rest0[:3]  # K1, K2, K3: read by this pass-1
                        fill = rest0[3:]
                        rest0 = []
                    else:
                        fill = []
                else:
                    k = len(thunks) // nq_cross + (1 if qb < len(thunks) % nq_cross else 0)
                    fill = thunks[:k]
                    thunks = thunks[k:]
                carry = emit_attn_hp(1, hp, qb, att_cross[qb], filler=fill, carry=carry, pre_fill=pre)
                if hp == CT - 1 and qb % 2 == 1:
                    # interleave the first self-branch hp0 blocks into the
                    # hp5 row: fills its exp-semaphore stalls with ready PE
                    # work and moves ACT work out of the ACT-bound self phase
                    carry = emit_attn_hp(
                        0, 0, qb // 2, att_self_early[qb // 2], carry=carry
                    )
            for th in thunks:
                th()

        # ---- self branch: qb-outer; deferred cross headouts as filler
        pending = [(1, qb, att_cross[qb]) for qb in range(nq_cross)]
        nq_self = N // qw
        for qb in range(nq_self):
            att_self = att_self_early[qb] if qb < 2 else alloc_attn(f"s{qb}")
            for hp in range(CT):
                if qb < 2 and hp == 0:
                    continue  # emitted inside the cross hp5 row
                carry = emit_attn_hp(0, hp, qb, att_self, carry=carry)
                if pending and len(drip) < 6:
                    drip.extend(headout_thunks(*pending.pop(0)))
            pending.append((0, qb, att_self))
        if carry is not None:
            carry()
        for th in drip:
            th()
        for blk in pending:
            emit_headout(*blk)
        if taps:
            nc.gpsimd.dma_start(dbg_ktt.ap(), ktt[0][:])
            nc.gpsimd.dma_start(dbg_qtt.ap(), qtt[0][:])
            nc.gpsimd.dma_start(dbg_v.ap(), v_sb[0][:])
            nc.gpsimd.dma_start(dbg_attn.ap(), att_self[0][:])

    nc.compile()
    return nc


_IDENT = np.eye(128, dtype=ml_dtypes.bfloat16)


def _prep_core_inputs(xb, yb, qkv_wt_bf, proj_wt_bf, proj_b):
    z = np.concatenate([xb, yb], axis=0)  # [S, C]
    zt = np.ascontiguousarray(z.T).astype(ml_dtypes.bfloat16)
    return {
        "z_t": zt,
        "qkv_wt": qkv_wt_bf,
        "proj_wt": proj_wt_bf,
        "proj_b": proj_b.reshape(1, -1).astype(np.float32),
        "ident": _IDENT,
    }


_NC_CACHE = {}


def kernel(x, y, qkv_w, proj_w, proj_b):
    x = np.asarray(x, dtype=np.float32)
    y = np.asarray(y, dtype=np.float32)
    qkv_w = np.asarray(qkv_w, dtype=np.float32)
    proj_w = np.asarray(proj_w, dtype=np.float32)
    proj_b = np.asarray(proj_b, dtype=np.float32)

    qkv_wt_bf = np.ascontiguousarray(qkv_w.T).astype(ml_dtypes.bfloat16)
    proj_wt_bf = np.ascontiguousarray(proj_w.T).astype(ml_dtypes.bfloat16)

    in_maps = [
        _prep_core_inputs(x[b], y[b], qkv_wt_bf, proj_wt_bf, proj_b)
        for b in range(x.shape[0])
    ]
    # cache the built/scheduled graph so repeated kernel() calls (e.g. a
    # warmup + measure harness) skip the Tile scheduling and reuse the
    # neuronxcc NEFF cache
    if "nc" not in _NC_CACHE:
        _NC_CACHE["nc"] = build_nc()
    nc = _NC_CACHE["nc"]
    res = bass_utils.run_bass_kernel_spmd(nc, in_maps, core_ids=list(range(len(in_maps))))
    x_out = np.stack(
        [res.results[b]["x_out"].astype(np.float32) for b in range(len(in_maps))]
    )
    y_out = np.stack(
        [res.results[b]["y_out"].astype(np.float32) for b in range(len(in_maps))]
    )
    return (x_out, y_out)


if __name__ == "__main__":
    import reference

    inputs = {k: np.asarray(v) for k, v in reference.setup_inputs().items()}
    out = kernel(**inputs)
    print("x_out", out[0].shape, "y_out", out[1].shape)
